# revision 4
# baseline (speedup 1.0000x reference)
"""nn_AdaptiveGaussianConv on 8 TRN2 NeuronCores (Bass/Tile).

Data-parallel over batch: one sample per core (B=8, n_cores=8); the
grouped conv and per-sample kernel generation are fully independent per
sample, so there are no collectives.

Per-core program (x [64, 384, 384] f32 -> out [64, 384, 384] f32):
  1. DMA-load x as bf16 (SWDGE inline cast) -- the whole sample stays
     resident in SBUF (18.9 MB), so x is read from HBM exactly once.
  2. Global average pool per channel: ScalarE activation(Copy) with
     accum_out gives per-partition sums; a ones-matmul reduces across
     partitions.
  3. MLP: h = silu(w1 @ pooled + b1); p = w2 @ h + b2;
     sigma = softplus(p0) (= ln(1+exp)), dx/dy = 2*tanh(p1/p2).
     The 7x7 Gaussian is separable: g = outer(ky, kx)/ (sum ky * sum kx),
     so the depthwise conv is two 7-tap 1-D convs, each expressed as a
     banded Toeplitz matmul. Band panels [128, 390] are generated
     on-device (iota -> subtract center -> square -> exp -> affine mask).
  4. Separable conv per channel: two banded matmuls with the DATA as the
     stationary operand; each matmul flips orientation
     ([h,w] -> [w,h'] -> [h',w']), so no explicit transposes are needed.
     The three 128-row input tiles accumulate into one PSUM bank: tile 0
     streams the full 384-wide output window with start=True, tiles 1/2
     add their 134-wide diagonal windows.
  5. PSUM->SBUF copies (VectorE mid, ScalarE out), DMA-out with
     bf16->f32 cast, 4 channels per DMA.
"""
import numpy as np

from concourse import bacc, tile, mybir
from concourse.bass_utils import run_bass_kernel_spmd

F32 = mybir.dt.float32
BF16 = mybir.dt.bfloat16

B = 8
C, H, W = 64, 384, 384
HW = H * W
T = 3           # 128-row tiles per image
P = 128
KW = 390        # band panel width (tile 0 streams the full output range)
CPR = 4         # channels per DMA region
NREG = C // CPR

# (band column slice, psum window slice) per input tile index.
# Tile 0 writes the FULL output window with start=True (the band mask
# provides zeros outside its diagonal) so later tiles accumulate into
# fully-initialized PSUM; tiles 1/2 add their 134-wide windows.
_WIN = [
    ((3, 387), (0, 384)),
    ((0, 134), (125, 259)),
    ((0, 131), (253, 384)),
]


def build_nc(num_devices=8):
    nc = bacc.Bacc("TRN2", target_bir_lowering=False, debug=False,
                   num_devices=num_devices)
    x_ext = nc.dram_tensor("x", [C, H, W], F32, kind="ExternalInput")
    w1_ext = nc.dram_tensor("w1t", [C, 16], F32, kind="ExternalInput")
    b1_ext = nc.dram_tensor("b1", [16, 1], F32, kind="ExternalInput")
    w2_ext = nc.dram_tensor("w2t", [16, 3], F32, kind="ExternalInput")
    b2_ext = nc.dram_tensor("b2", [1, 3], F32, kind="ExternalInput")
    out_ext = nc.dram_tensor("out", [C, H, W], F32, kind="ExternalOutput")

    with tile.TileContext(nc) as tc:
        with (
            tc.tile_pool(name="xdata", bufs=NREG) as xpool,
            tc.tile_pool(name="work", bufs=1) as wpool,
            tc.tile_pool(name="z", bufs=2) as zpool,
            tc.tile_pool(name="ostage", bufs=3) as opool,
            tc.tile_pool(name="psA", bufs=1, space="PSUM") as psA,
            tc.tile_pool(name="psB", bufs=1, space="PSUM") as psB,
            tc.tile_pool(name="psS", bufs=2, space="PSUM") as psS,
        ):
            # ---------- load x (f32 -> bf16 cast DMA), 4 channels/DMA ----
            regions = []
            for r in range(NREG):
                xr = xpool.tile([P, CPR * T * W], BF16, tag="xr")
                in_ap = x_ext.ap()[r * CPR:(r + 1) * CPR].rearrange(
                    "c (t p) w -> p c t w", p=P)
                out_ap = xr[:].rearrange("p (c t w) -> p c t w", c=CPR, t=T)
                nc.gpsimd.dma_start(out=out_ap, in_=in_ap)
                regions.append(xr)

            def xslice(c, t, lo, hi):
                r, ci = divmod(c, CPR)
                base = (ci * T + t) * W
                return regions[r][:, base + lo: base + hi]

            # ---------- constants ------------------------------------
            ones_col = wpool.tile([P, 1], F32)       # rhs for partition-sum
            nc.gpsimd.memset(ones_col[:], 1.0)
            ones_row = wpool.tile([1, P], F32)       # lhsT for broadcast
            nc.gpsimd.memset(ones_row[:], 1.0)

            w1T = wpool.tile([C, 16], F32)
            nc.gpsimd.dma_start(out=w1T[:], in_=w1_ext.ap())
            w2T = wpool.tile([16, 3], F32)
            nc.gpsimd.dma_start(out=w2T[:], in_=w2_ext.ap())
            b1_sb = wpool.tile([16, 1], F32)
            nc.gpsimd.dma_start(out=b1_sb[:], in_=b1_ext.ap())
            b2row = wpool.tile([1, 3], F32)
            nc.gpsimd.dma_start(out=b2row[:], in_=b2_ext.ap())

            # ---------- global average pool --------------------------
            acc = wpool.tile([P, C], F32)
            trash = wpool.tile([P, T * W], BF16)
            for c in range(C):
                nc.scalar.activation(
                    trash[:], xslice(c, 0, 0, T * W),
                    mybir.ActivationFunctionType.Copy,
                    accum_out=acc[:, c:c + 1])
            pooled_ps = psS.tile([C, 1], F32, tag="sm")
            nc.tensor.matmul(pooled_ps[:], acc[:], ones_col[:], start=True, stop=True)
            pooled_sb = wpool.tile([C, 1], F32)
            nc.vector.tensor_copy(pooled_sb[:], pooled_ps[:])

            # ---------- MLP ------------------------------------------
            h_ps = psS.tile([16, 1], F32, tag="sm")
            nc.tensor.matmul(h_ps[:], w1T[:], pooled_sb[:], start=True, stop=True)
            # silu(z) = z * sigmoid(z),  z = h_ps/HW + b1
            z_sb = wpool.tile([16, 1], F32)
            nc.scalar.activation(z_sb[:], h_ps[:],
                                 mybir.ActivationFunctionType.Identity,
                                 bias=b1_sb[:], scale=1.0 / float(HW))
            sgm = wpool.tile([16, 1], F32)
            nc.scalar.activation(sgm[:], h_ps[:],
                                 mybir.ActivationFunctionType.Sigmoid,
                                 bias=b1_sb[:], scale=1.0 / float(HW))
            h_sb = wpool.tile([16, 1], F32)
            nc.vector.tensor_tensor(h_sb[:], z_sb[:], sgm[:], mybir.AluOpType.mult)
            pT_ps = psS.tile([1, 3], F32, tag="sm")
            nc.tensor.matmul(pT_ps[:], h_sb[:], w2T[:], start=True, stop=True)
            pT = wpool.tile([1, 3], F32)
            nc.vector.tensor_tensor(pT[:], pT_ps[:], b2row[:], mybir.AluOpType.add)

            # ---------- scalar params on partition 0 ------------------
            sca = wpool.tile([1, 16], F32)  # scratch row of scalars

            def s(i):
                return sca[:, i:i + 1]
            # 0:sigma 1:s2 2:2s2 3:inv2s2 4:neg_inv2s2 5:thx 6:cx 7:mx
            # 8:thy 9:cy 10:my 11:Sy 12:Sx 13:S 14:invS 15:exp(p0)
            # tanh cluster first (shares the sigmoid table), then exp/ln
            nc.scalar.activation(s(5), pT[:, 1:2], mybir.ActivationFunctionType.Tanh)
            nc.vector.tensor_scalar(s(6), s(5), 2.0, 3.0,
                                    mybir.AluOpType.mult, mybir.AluOpType.add)
            nc.vector.tensor_scalar(s(7), s(5), -2.0, 3.0,
                                    mybir.AluOpType.mult, mybir.AluOpType.add)
            nc.scalar.activation(s(8), pT[:, 2:3], mybir.ActivationFunctionType.Tanh)
            nc.vector.tensor_scalar(s(9), s(8), 2.0, 3.0,
                                    mybir.AluOpType.mult, mybir.AluOpType.add)
            nc.vector.tensor_scalar(s(10), s(8), -2.0, 3.0,
                                    mybir.AluOpType.mult, mybir.AluOpType.add)
            # softplus(p0) = ln(1 + exp(p0))
            nc.scalar.activation(s(15), pT[:, 0:1], mybir.ActivationFunctionType.Exp)
            nc.vector.tensor_scalar(s(15), s(15), 1.0, None, mybir.AluOpType.add)
            nc.scalar.activation(s(0), s(15), mybir.ActivationFunctionType.Ln)
            nc.scalar.activation(s(1), s(0), mybir.ActivationFunctionType.Square)
            nc.vector.tensor_scalar(s(2), s(1), 2.0, None, mybir.AluOpType.mult)
            nc.vector.reciprocal(s(3), s(2))
            nc.vector.tensor_scalar(s(4), s(3), -1.0, None, mybir.AluOpType.mult)

            # 7-tap sums for normalization
            i7 = wpool.tile([1, 7], F32)
            nc.gpsimd.iota(i7[:], pattern=[[1, 7]], base=0, channel_multiplier=0,
                           allow_small_or_imprecise_dtypes=True)
            k7 = wpool.tile([1, 7], F32)
            for (c_ap, s_ap) in ((s(9), s(11)), (s(6), s(12))):
                nc.vector.tensor_scalar(k7[:], i7[:], c_ap, None,
                                        mybir.AluOpType.subtract)
                nc.scalar.activation(k7[:], k7[:], mybir.ActivationFunctionType.Square)
                nc.scalar.activation(k7[:], k7[:], mybir.ActivationFunctionType.Exp,
                                     scale=s(4))
                nc.vector.tensor_reduce(s_ap, k7[:], mybir.AxisListType.X,
                                        mybir.AluOpType.add)
            nc.vector.tensor_tensor(s(13), s(11), s(12), mybir.AluOpType.mult)
            nc.vector.reciprocal(s(14), s(13))

            # broadcast (neg_inv2s2, my, mx, invS) to all 128 partitions
            vec4 = wpool.tile([1, 4], F32)
            nc.vector.tensor_copy(vec4[:, 0:1], s(4))
            nc.vector.tensor_copy(vec4[:, 1:2], s(10))
            nc.vector.tensor_copy(vec4[:, 2:3], s(7))
            nc.vector.tensor_copy(vec4[:, 3:4], s(14))
            bc_ps = psS.tile([P, 4], F32, tag="sm")
            nc.tensor.matmul(bc_ps[:], ones_row[:], vec4[:], start=True, stop=True)
            bc = wpool.tile([P, 4], F32)
            nc.vector.tensor_copy(bc[:], bc_ps[:])

            # ---------- band matrices [128, KW] -----------------------
            dgrid = wpool.tile([P, KW], F32)
            nc.gpsimd.iota(dgrid[:], pattern=[[1, KW]], base=0, channel_multiplier=-1,
                           allow_small_or_imprecise_dtypes=True)
            bands = []
            for mcol, do_norm in ((1, True), (2, False)):  # my -> H band, mx -> W band
                g = wpool.tile([P, KW], F32, tag=f"bandf{mcol}")
                nc.vector.tensor_scalar(g[:], dgrid[:], bc[:, mcol:mcol + 1], None,
                                        mybir.AluOpType.subtract)
                nc.scalar.activation(g[:], g[:], mybir.ActivationFunctionType.Square)
                nc.scalar.activation(g[:], g[:], mybir.ActivationFunctionType.Exp,
                                     scale=bc[:, 0:1])
                nc.gpsimd.affine_select(g[:], g[:], pattern=[[1, KW]],
                                        compare_op=mybir.AluOpType.is_ge,
                                        fill=0.0, base=0, channel_multiplier=-1)
                nc.gpsimd.affine_select(g[:], g[:], pattern=[[-1, KW]],
                                        compare_op=mybir.AluOpType.is_ge,
                                        fill=0.0, base=6, channel_multiplier=1)
                if do_norm:
                    nc.vector.tensor_scalar(g[:], g[:], bc[:, 3:4], None,
                                            mybir.AluOpType.mult)
                gb = wpool.tile([P, KW], BF16, tag=f"band{mcol}")
                nc.vector.tensor_copy(gb[:], g[:])
                bands.append(gb)
            bandH, bandW = bands

            # ---------- separable conv, per channel -------------------
            # 3-bank PSUM tiles: each 128-block's matmul group lands in
            # its own 512-f32-aligned sub-bank; one batched PSUM->SBUF
            # copy per channel per pass.
            for c in range(C):
                # pass 1: contract h -> ZhT [w, h'] per 128-col block
                ps1 = psA.tile([P, T, 512], F32, tag="ps1")
                for wb in range(T):
                    for t in range(T):
                        (b0, b1e), (o0, o1) = _WIN[t]
                        nc.tensor.matmul(
                            ps1[:, wb, o0:o1],
                            xslice(c, t, wb * P, (wb + 1) * P),
                            bandH[:, b0:b1e],
                            start=(t == 0), stop=(t == T - 1))
                zb = zpool.tile([P, T * W], BF16, tag="zt")
                nc.vector.tensor_copy(
                    zb[:].rearrange("p (t w) -> p t w", t=T),
                    ps1[:, :, 0:W])
                # pass 2: contract w -> out [h', w'] per 128-row block
                r, ci = divmod(c, CPR)
                if ci == 0:
                    ost = opool.tile([P, CPR * T * W], BF16, tag="ost")
                ps2 = psB.tile([P, T, 512], F32, tag="ps2")
                for hb in range(T):
                    for t2 in range(T):
                        (b0, b1e), (o0, o1) = _WIN[t2]
                        nc.tensor.matmul(
                            ps2[:, hb, o0:o1],
                            zb[:, t2 * W + hb * P: t2 * W + (hb + 1) * P],
                            bandW[:, b0:b1e],
                            start=(t2 == 0), stop=(t2 == T - 1))
                nc.scalar.copy(
                    ost[:, ci * T * W:(ci + 1) * T * W].rearrange(
                        "p (t w) -> p t w", t=T),
                    ps2[:, :, 0:W])
                if ci == CPR - 1:
                    out_ap = out_ext.ap()[r * CPR:(r + 1) * CPR].rearrange(
                        "c (t p) w -> p c t w", p=P)
                    in_ap = ost[:].rearrange("p (c t w) -> p c t w", c=CPR, t=T)
                    nc.gpsimd.dma_start(out=out_ap, in_=in_ap)

    nc.compile()
    return nc


_NC = None
LAST_EXEC_TIME_NS = None
LAST_RESULTS = None


def _get_nc():
    global _NC
    if _NC is None:
        _NC = build_nc(num_devices=B)
    return _NC


def kernel(x, w1, b1, w2, b2):
    """Full inputs in, full output out; shards batch across 8 cores."""
    global LAST_EXEC_TIME_NS, LAST_RESULTS
    x = np.ascontiguousarray(x, dtype=np.float32)
    w1t = np.ascontiguousarray(w1.T, dtype=np.float32)
    b1c = np.ascontiguousarray(np.asarray(b1, dtype=np.float32).reshape(16, 1))
    w2t = np.ascontiguousarray(w2.T, dtype=np.float32)
    b2r = np.ascontiguousarray(np.asarray(b2, dtype=np.float32).reshape(1, 3))
    in_maps = [
        {"x": x[i], "w1t": w1t, "b1": b1c, "w2t": w2t, "b2": b2r}
        for i in range(B)
    ]
    nc = _get_nc()
    try:
        res = run_bass_kernel_spmd(nc, in_maps, core_ids=list(range(B)), trace=True)
    except Exception:
        res = run_bass_kernel_spmd(nc, in_maps, core_ids=list(range(B)), trace=False)
    LAST_EXEC_TIME_NS = res.exec_time_ns
    LAST_RESULTS = res
    out = np.stack([res.results[i]["out"] for i in range(B)], axis=0)
    return out.astype(np.float32, copy=False)


# revision 6
# speedup vs baseline: 1.2195x; 1.2195x over previous
"""nn_AdaptiveGaussianConv on 8 TRN2 NeuronCores (Bass/Tile).

Data-parallel over batch: one sample per core (B=8, n_cores=8); the
grouped conv and per-sample kernel generation are fully independent per
sample, so there are no collectives.

Per-core program (x [64, 384, 384] f32 -> out [64, 384, 384] f32):
  1. DMA-load x as bf16 (SWDGE inline cast) -- the whole sample stays
     resident in SBUF (18.9 MB), so x is read from HBM exactly once.
  2. Global average pool per channel: ScalarE activation(Copy) with
     accum_out gives per-partition sums; a ones-matmul reduces across
     partitions.
  3. MLP: h = silu(w1 @ pooled + b1); p = w2 @ h + b2;
     sigma = softplus(p0) (= ln(1+exp)), dx/dy = 2*tanh(p1/p2).
     The 7x7 Gaussian is separable: g = outer(ky, kx)/ (sum ky * sum kx),
     so the depthwise conv is two 7-tap 1-D convs, each expressed as a
     banded Toeplitz matmul. Band panels [128, 390] are generated
     on-device (iota -> subtract center -> square -> exp -> affine mask).
  4. Separable conv per channel: two banded matmuls with the DATA as the
     stationary operand; each matmul flips orientation
     ([h,w] -> [w,h'] -> [h',w']), so no explicit transposes are needed.
     The three 128-row input tiles accumulate into one PSUM bank: tile 0
     streams the full 384-wide output window with start=True, tiles 1/2
     add their 134-wide diagonal windows.
  5. PSUM->SBUF copies (VectorE mid, ScalarE out), DMA-out with
     bf16->f32 cast, 4 channels per DMA.
"""
import numpy as np

from concourse import bacc, tile, mybir
from concourse.bass_utils import run_bass_kernel_spmd

F32 = mybir.dt.float32
BF16 = mybir.dt.bfloat16

B = 8
C, H, W = 64, 384, 384
HW = H * W
T = 3           # 128-row tiles per image
P = 128
KW = 390        # band panel width (tile 0 streams the full output range)
CPR = 4         # channels per DMA region
NREG = C // CPR

# (band column slice, psum window slice) per input tile index.
# Tile 0 writes the FULL output window with start=True (the band mask
# provides zeros outside its diagonal) so later tiles accumulate into
# fully-initialized PSUM; tiles 1/2 add their 134-wide windows.
_WIN = [
    ((3, 387), (0, 384)),
    ((0, 134), (125, 259)),
    ((0, 131), (253, 384)),
]


def build_nc(num_devices=8):
    nc = bacc.Bacc("TRN2", target_bir_lowering=False, debug=False,
                   num_devices=num_devices)
    x_ext = nc.dram_tensor("x", [C, H, W], F32, kind="ExternalInput")
    w1_ext = nc.dram_tensor("w1t", [C, 16], F32, kind="ExternalInput")
    b1_ext = nc.dram_tensor("b1", [16, 1], F32, kind="ExternalInput")
    w2_ext = nc.dram_tensor("w2t", [16, 3], F32, kind="ExternalInput")
    b2_ext = nc.dram_tensor("b2", [1, 3], F32, kind="ExternalInput")
    out_ext = nc.dram_tensor("out", [C, H, W], F32, kind="ExternalOutput")

    with tile.TileContext(nc) as tc:
        with (
            tc.tile_pool(name="xdata", bufs=NREG) as xpool,
            tc.tile_pool(name="work", bufs=1) as wpool,
            tc.tile_pool(name="z", bufs=2) as zpool,
            tc.tile_pool(name="ostage", bufs=3) as opool,
            tc.tile_pool(name="psA", bufs=3, space="PSUM") as psA,
            tc.tile_pool(name="psB", bufs=1, space="PSUM") as psB,
            tc.tile_pool(name="psS", bufs=2, space="PSUM") as psS,
        ):
            # ---------- load x (f32 -> bf16 cast DMA), 4 channels/DMA ----
            regions = []
            for r in range(NREG):
                xr = xpool.tile([P, CPR * T * W], BF16, tag="xr")
                in_ap = x_ext.ap()[r * CPR:(r + 1) * CPR].rearrange(
                    "c (t p) w -> p c t w", p=P)
                out_ap = xr[:].rearrange("p (c t w) -> p c t w", c=CPR, t=T)
                nc.gpsimd.dma_start(out=out_ap, in_=in_ap)
                regions.append(xr)

            def xslice(c, t, lo, hi):
                r, ci = divmod(c, CPR)
                base = (ci * T + t) * W
                return regions[r][:, base + lo: base + hi]

            # ---------- constants ------------------------------------
            ones_col = wpool.tile([P, 1], F32)       # rhs for partition-sum
            nc.gpsimd.memset(ones_col[:], 1.0)
            ones_row = wpool.tile([1, P], F32)       # lhsT for broadcast
            nc.gpsimd.memset(ones_row[:], 1.0)

            w1T = wpool.tile([C, 16], F32)
            nc.gpsimd.dma_start(out=w1T[:], in_=w1_ext.ap())
            w2T = wpool.tile([16, 3], F32)
            nc.gpsimd.dma_start(out=w2T[:], in_=w2_ext.ap())
            b1_sb = wpool.tile([16, 1], F32)
            nc.gpsimd.dma_start(out=b1_sb[:], in_=b1_ext.ap())
            b2row = wpool.tile([1, 3], F32)
            nc.gpsimd.dma_start(out=b2row[:], in_=b2_ext.ap())

            # ---------- global average pool --------------------------
            acc = wpool.tile([P, C], F32)
            trash = wpool.tile([P, T * W], BF16)
            for c in range(C):
                nc.scalar.activation(
                    trash[:], xslice(c, 0, 0, T * W),
                    mybir.ActivationFunctionType.Copy,
                    accum_out=acc[:, c:c + 1])
            pooled_ps = psS.tile([C, 1], F32, tag="sm")
            nc.tensor.matmul(pooled_ps[:], acc[:], ones_col[:], start=True, stop=True)
            pooled_sb = wpool.tile([C, 1], F32)
            nc.vector.tensor_copy(pooled_sb[:], pooled_ps[:])

            # ---------- MLP ------------------------------------------
            h_ps = psS.tile([16, 1], F32, tag="sm")
            nc.tensor.matmul(h_ps[:], w1T[:], pooled_sb[:], start=True, stop=True)
            # silu(z) = z * sigmoid(z),  z = h_ps/HW + b1
            z_sb = wpool.tile([16, 1], F32)
            nc.scalar.activation(z_sb[:], h_ps[:],
                                 mybir.ActivationFunctionType.Identity,
                                 bias=b1_sb[:], scale=1.0 / float(HW))
            sgm = wpool.tile([16, 1], F32)
            nc.scalar.activation(sgm[:], h_ps[:],
                                 mybir.ActivationFunctionType.Sigmoid,
                                 bias=b1_sb[:], scale=1.0 / float(HW))
            h_sb = wpool.tile([16, 1], F32)
            nc.vector.tensor_tensor(h_sb[:], z_sb[:], sgm[:], mybir.AluOpType.mult)
            pT_ps = psS.tile([1, 3], F32, tag="sm")
            nc.tensor.matmul(pT_ps[:], h_sb[:], w2T[:], start=True, stop=True)
            pT = wpool.tile([1, 3], F32)
            nc.vector.tensor_tensor(pT[:], pT_ps[:], b2row[:], mybir.AluOpType.add)

            # ---------- scalar params on partition 0 ------------------
            sca = wpool.tile([1, 16], F32)  # scratch row of scalars

            def s(i):
                return sca[:, i:i + 1]
            # 0:sigma 1:s2 2:2s2 3:inv2s2 4:neg_inv2s2 5:thx 6:cx 7:mx
            # 8:thy 9:cy 10:my 11:Sy 12:Sx 13:S 14:invS 15:exp(p0)
            # tanh cluster first (shares the sigmoid table), then exp/ln
            nc.scalar.activation(s(5), pT[:, 1:2], mybir.ActivationFunctionType.Tanh)
            nc.vector.tensor_scalar(s(6), s(5), 2.0, 3.0,
                                    mybir.AluOpType.mult, mybir.AluOpType.add)
            nc.vector.tensor_scalar(s(7), s(5), -2.0, 3.0,
                                    mybir.AluOpType.mult, mybir.AluOpType.add)
            nc.scalar.activation(s(8), pT[:, 2:3], mybir.ActivationFunctionType.Tanh)
            nc.vector.tensor_scalar(s(9), s(8), 2.0, 3.0,
                                    mybir.AluOpType.mult, mybir.AluOpType.add)
            nc.vector.tensor_scalar(s(10), s(8), -2.0, 3.0,
                                    mybir.AluOpType.mult, mybir.AluOpType.add)
            # softplus(p0) = ln(1 + exp(p0))
            nc.scalar.activation(s(15), pT[:, 0:1], mybir.ActivationFunctionType.Exp)
            nc.vector.tensor_scalar(s(15), s(15), 1.0, None, mybir.AluOpType.add)
            nc.scalar.activation(s(0), s(15), mybir.ActivationFunctionType.Ln)
            nc.scalar.activation(s(1), s(0), mybir.ActivationFunctionType.Square)
            nc.vector.tensor_scalar(s(2), s(1), 2.0, None, mybir.AluOpType.mult)
            nc.vector.reciprocal(s(3), s(2))
            nc.vector.tensor_scalar(s(4), s(3), -1.0, None, mybir.AluOpType.mult)

            # 7-tap sums for normalization
            i7 = wpool.tile([1, 7], F32)
            nc.gpsimd.iota(i7[:], pattern=[[1, 7]], base=0, channel_multiplier=0,
                           allow_small_or_imprecise_dtypes=True)
            k7 = wpool.tile([1, 7], F32)
            for (c_ap, s_ap) in ((s(9), s(11)), (s(6), s(12))):
                nc.vector.tensor_scalar(k7[:], i7[:], c_ap, None,
                                        mybir.AluOpType.subtract)
                nc.scalar.activation(k7[:], k7[:], mybir.ActivationFunctionType.Square)
                nc.scalar.activation(k7[:], k7[:], mybir.ActivationFunctionType.Exp,
                                     scale=s(4))
                nc.vector.tensor_reduce(s_ap, k7[:], mybir.AxisListType.X,
                                        mybir.AluOpType.add)
            nc.vector.tensor_tensor(s(13), s(11), s(12), mybir.AluOpType.mult)
            nc.vector.reciprocal(s(14), s(13))

            # broadcast (neg_inv2s2, my, mx, invS) to all 128 partitions
            vec4 = wpool.tile([1, 4], F32)
            nc.vector.tensor_copy(vec4[:, 0:1], s(4))
            nc.vector.tensor_copy(vec4[:, 1:2], s(10))
            nc.vector.tensor_copy(vec4[:, 2:3], s(7))
            nc.vector.tensor_copy(vec4[:, 3:4], s(14))
            bc_ps = psS.tile([P, 4], F32, tag="sm")
            nc.tensor.matmul(bc_ps[:], ones_row[:], vec4[:], start=True, stop=True)
            bc = wpool.tile([P, 4], F32)
            nc.vector.tensor_copy(bc[:], bc_ps[:])

            # ---------- band matrices [128, KW] -----------------------
            dgrid = wpool.tile([P, KW], F32)
            nc.gpsimd.iota(dgrid[:], pattern=[[1, KW]], base=0, channel_multiplier=-1,
                           allow_small_or_imprecise_dtypes=True)
            bands = []
            for mcol, do_norm in ((1, True), (2, False)):  # my -> H band, mx -> W band
                g = wpool.tile([P, KW], F32, tag=f"bandf{mcol}")
                nc.vector.tensor_scalar(g[:], dgrid[:], bc[:, mcol:mcol + 1], None,
                                        mybir.AluOpType.subtract)
                nc.scalar.activation(g[:], g[:], mybir.ActivationFunctionType.Square)
                nc.scalar.activation(g[:], g[:], mybir.ActivationFunctionType.Exp,
                                     scale=bc[:, 0:1])
                nc.gpsimd.affine_select(g[:], g[:], pattern=[[1, KW]],
                                        compare_op=mybir.AluOpType.is_ge,
                                        fill=0.0, base=0, channel_multiplier=-1)
                nc.gpsimd.affine_select(g[:], g[:], pattern=[[-1, KW]],
                                        compare_op=mybir.AluOpType.is_ge,
                                        fill=0.0, base=6, channel_multiplier=1)
                if do_norm:
                    nc.vector.tensor_scalar(g[:], g[:], bc[:, 3:4], None,
                                            mybir.AluOpType.mult)
                gb = wpool.tile([P, KW], BF16, tag=f"band{mcol}")
                nc.vector.tensor_copy(gb[:], g[:])
                bands.append(gb)
            bandH, bandW = bands

            # ---------- separable conv, per channel -------------------
            # 3-bank PSUM tiles: each 128-block's matmul group lands in
            # its own 512-f32-aligned sub-bank; one batched PSUM->SBUF
            # copy per channel per pass.
            for c in range(C):
                # pass 1: contract h -> ZhT [w, h'] per 128-col block
                zb = zpool.tile([P, T * W], BF16, tag="zt")
                for wb in range(T):
                    ps1 = psA.tile([P, W], F32, tag="ps1")
                    for t in range(T):
                        (b0, b1e), (o0, o1) = _WIN[t]
                        nc.tensor.matmul(
                            ps1[:, o0:o1],
                            xslice(c, t, wb * P, (wb + 1) * P),
                            bandH[:, b0:b1e],
                            start=(t == 0), stop=(t == T - 1))
                    nc.vector.tensor_copy(zb[:, wb * W:(wb + 1) * W], ps1[:])
                # pass 2: contract w -> out [h', w'] per 128-row block
                r, ci = divmod(c, CPR)
                if ci == 0:
                    ost = opool.tile([P, CPR * T * W], BF16, tag="ost")
                ps2 = psB.tile([P, T, 512], F32, tag="ps2")
                for hb in range(T):
                    for t2 in range(T):
                        (b0, b1e), (o0, o1) = _WIN[t2]
                        nc.tensor.matmul(
                            ps2[:, hb, o0:o1],
                            zb[:, t2 * W + hb * P: t2 * W + (hb + 1) * P],
                            bandW[:, b0:b1e],
                            start=(t2 == 0), stop=(t2 == T - 1))
                nc.scalar.copy(
                    ost[:, ci * T * W:(ci + 1) * T * W].rearrange(
                        "p (t w) -> p t w", t=T),
                    ps2[:, :, 0:W])
                if ci == CPR - 1:
                    out_ap = out_ext.ap()[r * CPR:(r + 1) * CPR].rearrange(
                        "c (t p) w -> p c t w", p=P)
                    in_ap = ost[:].rearrange("p (c t w) -> p c t w", c=CPR, t=T)
                    nc.gpsimd.dma_start(out=out_ap, in_=in_ap)

    nc.compile()
    return nc


_NC = None
LAST_EXEC_TIME_NS = None
LAST_RESULTS = None


def _get_nc():
    global _NC
    if _NC is None:
        _NC = build_nc(num_devices=B)
    return _NC


def kernel(x, w1, b1, w2, b2):
    """Full inputs in, full output out; shards batch across 8 cores."""
    global LAST_EXEC_TIME_NS, LAST_RESULTS
    x = np.ascontiguousarray(x, dtype=np.float32)
    w1t = np.ascontiguousarray(w1.T, dtype=np.float32)
    b1c = np.ascontiguousarray(np.asarray(b1, dtype=np.float32).reshape(16, 1))
    w2t = np.ascontiguousarray(w2.T, dtype=np.float32)
    b2r = np.ascontiguousarray(np.asarray(b2, dtype=np.float32).reshape(1, 3))
    in_maps = [
        {"x": x[i], "w1t": w1t, "b1": b1c, "w2t": w2t, "b2": b2r}
        for i in range(B)
    ]
    nc = _get_nc()
    try:
        res = run_bass_kernel_spmd(nc, in_maps, core_ids=list(range(B)), trace=True)
    except Exception:
        res = run_bass_kernel_spmd(nc, in_maps, core_ids=list(range(B)), trace=False)
    LAST_EXEC_TIME_NS = res.exec_time_ns
    LAST_RESULTS = res
    out = np.stack([res.results[i]["out"] for i in range(B)], axis=0)
    return out.astype(np.float32, copy=False)


# revision 8
# speedup vs baseline: 1.3889x; 1.1389x over previous
"""nn_AdaptiveGaussianConv on 8 TRN2 NeuronCores (Bass/Tile).

Data-parallel over batch: one sample per core (B=8, n_cores=8); the
grouped conv and per-sample kernel generation are fully independent per
sample, so there are no collectives.

Per-core program (x [64, 384, 384] f32 -> out [64, 384, 384] f32):
  1. DMA-load x as bf16 (SWDGE inline cast) -- the whole sample stays
     resident in SBUF (18.9 MB), so x is read from HBM exactly once.
  2. Global average pool per channel: ScalarE activation(Copy) with
     accum_out gives per-partition sums; a ones-matmul reduces across
     partitions.
  3. MLP: h = silu(w1 @ pooled + b1); p = w2 @ h + b2;
     sigma = softplus(p0) (= ln(1+exp)), dx/dy = 2*tanh(p1/p2).
     The 7x7 Gaussian is separable: g = outer(ky, kx)/ (sum ky * sum kx),
     so the depthwise conv is two 7-tap 1-D convs, each expressed as a
     banded Toeplitz matmul. Band panels [128, 390] are generated
     on-device (iota -> subtract center -> square -> exp -> affine mask).
  4. Separable conv per channel: two banded matmuls with the DATA as the
     stationary operand; each matmul flips orientation
     ([h,w] -> [w,h'] -> [h',w']), so no explicit transposes are needed.
     The three 128-row input tiles accumulate into one PSUM bank: tile 0
     streams the full 384-wide output window with start=True, tiles 1/2
     add their 134-wide diagonal windows.
  5. PSUM->SBUF copies (VectorE mid, ScalarE out), DMA-out with
     bf16->f32 cast, 4 channels per DMA.
"""
import numpy as np

from concourse import bacc, tile, mybir
from concourse.bass_utils import run_bass_kernel_spmd

F32 = mybir.dt.float32
BF16 = mybir.dt.bfloat16

B = 8
C, H, W = 64, 384, 384
HW = H * W
T = 3           # 128-row tiles per image
P = 128
KW = 390        # band panel width (tile 0 streams the full output range)
CPR = 4         # channels per DMA region
NREG = C // CPR

# (band column slice, psum window slice) per input tile index.
# Tile 0 writes the FULL output window with start=True (the band mask
# provides zeros outside its diagonal) so later tiles accumulate into
# fully-initialized PSUM; tiles 1/2 add their 134-wide windows.
_WIN = [
    ((3, 387), (0, 384)),
    ((0, 134), (125, 259)),
    ((0, 131), (253, 384)),
]


def build_nc(num_devices=8):
    nc = bacc.Bacc("TRN2", target_bir_lowering=False, debug=False,
                   num_devices=num_devices)
    x_ext = nc.dram_tensor("x", [C, H, W], F32, kind="ExternalInput")
    w1_ext = nc.dram_tensor("w1t", [C, 16], F32, kind="ExternalInput")
    b1_ext = nc.dram_tensor("b1", [16, 1], F32, kind="ExternalInput")
    w2_ext = nc.dram_tensor("w2t", [16, 3], F32, kind="ExternalInput")
    b2_ext = nc.dram_tensor("b2", [1, 3], F32, kind="ExternalInput")
    out_ext = nc.dram_tensor("out", [C, H, W], F32, kind="ExternalOutput")

    with tile.TileContext(nc) as tc:
        with (
            tc.tile_pool(name="xdata", bufs=NREG) as xpool,
            tc.tile_pool(name="work", bufs=1) as wpool,
            tc.tile_pool(name="z", bufs=2) as zpool,
            tc.tile_pool(name="ostage", bufs=3) as opool,
            tc.tile_pool(name="psA", bufs=3, space="PSUM") as psA,
            tc.tile_pool(name="psB", bufs=3, space="PSUM") as psB,
            tc.tile_pool(name="psS", bufs=2, space="PSUM") as psS,
        ):
            # ---------- load x (f32 -> bf16 cast DMA), 4 channels/DMA ----
            regions = []
            for r in range(NREG):
                xr = xpool.tile([P, CPR * T * W], BF16, tag="xr")
                in_ap = x_ext.ap()[r * CPR:(r + 1) * CPR].rearrange(
                    "c (t p) w -> p c t w", p=P)
                out_ap = xr[:].rearrange("p (c t w) -> p c t w", c=CPR, t=T)
                nc.gpsimd.dma_start(out=out_ap, in_=in_ap)
                regions.append(xr)

            def xslice(c, t, lo, hi):
                r, ci = divmod(c, CPR)
                base = (ci * T + t) * W
                return regions[r][:, base + lo: base + hi]

            # ---------- constants ------------------------------------
            ones_col = wpool.tile([P, 1], F32)       # rhs for partition-sum
            nc.gpsimd.memset(ones_col[:], 1.0)
            ones_row = wpool.tile([1, P], F32)       # lhsT for broadcast
            nc.gpsimd.memset(ones_row[:], 1.0)

            w1T = wpool.tile([C, 16], F32)
            nc.gpsimd.dma_start(out=w1T[:], in_=w1_ext.ap())
            w2T = wpool.tile([16, 3], F32)
            nc.gpsimd.dma_start(out=w2T[:], in_=w2_ext.ap())
            b1_sb = wpool.tile([16, 1], F32)
            nc.gpsimd.dma_start(out=b1_sb[:], in_=b1_ext.ap())
            b2row = wpool.tile([1, 3], F32)
            nc.gpsimd.dma_start(out=b2row[:], in_=b2_ext.ap())

            # ---------- global average pool --------------------------
            acc = wpool.tile([P, C], F32)
            trash = wpool.tile([P, T * W], BF16)
            for c in range(C):
                nc.scalar.activation(
                    trash[:], xslice(c, 0, 0, T * W),
                    mybir.ActivationFunctionType.Copy,
                    accum_out=acc[:, c:c + 1])
            pooled_ps = psS.tile([C, 1], F32, tag="sm")
            nc.tensor.matmul(pooled_ps[:], acc[:], ones_col[:], start=True, stop=True)
            pooled_sb = wpool.tile([C, 1], F32)
            nc.vector.tensor_copy(pooled_sb[:], pooled_ps[:])

            # ---------- MLP ------------------------------------------
            h_ps = psS.tile([16, 1], F32, tag="sm")
            nc.tensor.matmul(h_ps[:], w1T[:], pooled_sb[:], start=True, stop=True)
            # silu(z) = z * sigmoid(z),  z = h_ps/HW + b1
            z_sb = wpool.tile([16, 1], F32)
            nc.scalar.activation(z_sb[:], h_ps[:],
                                 mybir.ActivationFunctionType.Identity,
                                 bias=b1_sb[:], scale=1.0 / float(HW))
            sgm = wpool.tile([16, 1], F32)
            nc.scalar.activation(sgm[:], h_ps[:],
                                 mybir.ActivationFunctionType.Sigmoid,
                                 bias=b1_sb[:], scale=1.0 / float(HW))
            h_sb = wpool.tile([16, 1], F32)
            nc.vector.tensor_tensor(h_sb[:], z_sb[:], sgm[:], mybir.AluOpType.mult)
            pT_ps = psS.tile([1, 3], F32, tag="sm")
            nc.tensor.matmul(pT_ps[:], h_sb[:], w2T[:], start=True, stop=True)
            pT = wpool.tile([1, 3], F32)
            nc.vector.tensor_tensor(pT[:], pT_ps[:], b2row[:], mybir.AluOpType.add)

            # ---------- scalar params on partition 0 ------------------
            sca = wpool.tile([1, 16], F32)  # scratch row of scalars

            def s(i):
                return sca[:, i:i + 1]
            # 0:sigma 1:s2 2:2s2 3:inv2s2 4:neg_inv2s2 5:thx 6:cx 7:mx
            # 8:thy 9:cy 10:my 11:Sy 12:Sx 13:S 14:invS 15:exp(p0)
            # tanh cluster first (shares the sigmoid table), then exp/ln
            nc.scalar.activation(s(5), pT[:, 1:2], mybir.ActivationFunctionType.Tanh)
            nc.vector.tensor_scalar(s(6), s(5), 2.0, 3.0,
                                    mybir.AluOpType.mult, mybir.AluOpType.add)
            nc.vector.tensor_scalar(s(7), s(5), -2.0, 3.0,
                                    mybir.AluOpType.mult, mybir.AluOpType.add)
            nc.scalar.activation(s(8), pT[:, 2:3], mybir.ActivationFunctionType.Tanh)
            nc.vector.tensor_scalar(s(9), s(8), 2.0, 3.0,
                                    mybir.AluOpType.mult, mybir.AluOpType.add)
            nc.vector.tensor_scalar(s(10), s(8), -2.0, 3.0,
                                    mybir.AluOpType.mult, mybir.AluOpType.add)
            # softplus(p0) = ln(1 + exp(p0))
            nc.scalar.activation(s(15), pT[:, 0:1], mybir.ActivationFunctionType.Exp)
            nc.vector.tensor_scalar(s(15), s(15), 1.0, None, mybir.AluOpType.add)
            nc.scalar.activation(s(0), s(15), mybir.ActivationFunctionType.Ln)
            nc.scalar.activation(s(1), s(0), mybir.ActivationFunctionType.Square)
            nc.vector.tensor_scalar(s(2), s(1), 2.0, None, mybir.AluOpType.mult)
            nc.vector.reciprocal(s(3), s(2))
            nc.vector.tensor_scalar(s(4), s(3), -1.0, None, mybir.AluOpType.mult)

            # 7-tap sums for normalization
            i7 = wpool.tile([1, 7], F32)
            nc.gpsimd.iota(i7[:], pattern=[[1, 7]], base=0, channel_multiplier=0,
                           allow_small_or_imprecise_dtypes=True)
            k7 = wpool.tile([1, 7], F32)
            for (c_ap, s_ap) in ((s(9), s(11)), (s(6), s(12))):
                nc.vector.tensor_scalar(k7[:], i7[:], c_ap, None,
                                        mybir.AluOpType.subtract)
                nc.scalar.activation(k7[:], k7[:], mybir.ActivationFunctionType.Square)
                nc.scalar.activation(k7[:], k7[:], mybir.ActivationFunctionType.Exp,
                                     scale=s(4))
                nc.vector.tensor_reduce(s_ap, k7[:], mybir.AxisListType.X,
                                        mybir.AluOpType.add)
            nc.vector.tensor_tensor(s(13), s(11), s(12), mybir.AluOpType.mult)
            nc.vector.reciprocal(s(14), s(13))

            # broadcast (neg_inv2s2, my, mx, invS) to all 128 partitions
            vec4 = wpool.tile([1, 4], F32)
            nc.vector.tensor_copy(vec4[:, 0:1], s(4))
            nc.vector.tensor_copy(vec4[:, 1:2], s(10))
            nc.vector.tensor_copy(vec4[:, 2:3], s(7))
            nc.vector.tensor_copy(vec4[:, 3:4], s(14))
            bc_ps = psS.tile([P, 4], F32, tag="sm")
            nc.tensor.matmul(bc_ps[:], ones_row[:], vec4[:], start=True, stop=True)
            bc = wpool.tile([P, 4], F32)
            nc.vector.tensor_copy(bc[:], bc_ps[:])

            # ---------- band matrices [128, KW] -----------------------
            dgrid = wpool.tile([P, KW], F32)
            nc.gpsimd.iota(dgrid[:], pattern=[[1, KW]], base=0, channel_multiplier=-1,
                           allow_small_or_imprecise_dtypes=True)
            bands = []
            for mcol, do_norm in ((1, True), (2, False)):  # my -> H band, mx -> W band
                g = wpool.tile([P, KW], F32, tag=f"bandf{mcol}")
                nc.vector.tensor_scalar(g[:], dgrid[:], bc[:, mcol:mcol + 1], None,
                                        mybir.AluOpType.subtract)
                nc.scalar.activation(g[:], g[:], mybir.ActivationFunctionType.Square)
                nc.scalar.activation(g[:], g[:], mybir.ActivationFunctionType.Exp,
                                     scale=bc[:, 0:1])
                nc.gpsimd.affine_select(g[:], g[:], pattern=[[1, KW]],
                                        compare_op=mybir.AluOpType.is_ge,
                                        fill=0.0, base=0, channel_multiplier=-1)
                nc.gpsimd.affine_select(g[:], g[:], pattern=[[-1, KW]],
                                        compare_op=mybir.AluOpType.is_ge,
                                        fill=0.0, base=6, channel_multiplier=1)
                if do_norm:
                    nc.vector.tensor_scalar(g[:], g[:], bc[:, 3:4], None,
                                            mybir.AluOpType.mult)
                gb = wpool.tile([P, KW], BF16, tag=f"band{mcol}")
                nc.vector.tensor_copy(gb[:], g[:])
                bands.append(gb)
            bandH, bandW = bands

            # ---------- separable conv, per channel -------------------
            # 3-bank PSUM tiles: each 128-block's matmul group lands in
            # its own 512-f32-aligned sub-bank; one batched PSUM->SBUF
            # copy per channel per pass.
            for c in range(C):
                # pass 1: contract h -> ZhT [w, h'] per 128-col block
                zb = zpool.tile([P, T * W], BF16, tag="zt")
                for wb in range(T):
                    ps1 = psA.tile([P, W], F32, tag="ps1")
                    for t in range(T):
                        (b0, b1e), (o0, o1) = _WIN[t]
                        nc.tensor.matmul(
                            ps1[:, o0:o1],
                            xslice(c, t, wb * P, (wb + 1) * P),
                            bandH[:, b0:b1e],
                            start=(t == 0), stop=(t == T - 1))
                    nc.vector.tensor_copy(zb[:, wb * W:(wb + 1) * W], ps1[:])
                # pass 2: contract w -> out [h', w'] per 128-row block
                r, ci = divmod(c, CPR)
                if ci == 0:
                    ost = opool.tile([P, CPR * T * W], BF16, tag="ost")
                for hb in range(T):
                    ps2 = psB.tile([P, W], F32, tag="ps2")
                    for t2 in range(T):
                        (b0, b1e), (o0, o1) = _WIN[t2]
                        nc.tensor.matmul(
                            ps2[:, o0:o1],
                            zb[:, t2 * W + hb * P: t2 * W + (hb + 1) * P],
                            bandW[:, b0:b1e],
                            start=(t2 == 0), stop=(t2 == T - 1))
                    nc.scalar.copy(ost[:, (ci * T + hb) * W:(ci * T + hb + 1) * W],
                                   ps2[:])
                if ci == CPR - 1:
                    out_ap = out_ext.ap()[r * CPR:(r + 1) * CPR].rearrange(
                        "c (t p) w -> p c t w", p=P)
                    in_ap = ost[:].rearrange("p (c t w) -> p c t w", c=CPR, t=T)
                    nc.gpsimd.dma_start(out=out_ap, in_=in_ap)

    nc.compile()
    return nc


_NC = None
LAST_EXEC_TIME_NS = None
LAST_RESULTS = None


def _get_nc():
    global _NC
    if _NC is None:
        _NC = build_nc(num_devices=B)
    return _NC


def kernel(x, w1, b1, w2, b2):
    """Full inputs in, full output out; shards batch across 8 cores."""
    global LAST_EXEC_TIME_NS, LAST_RESULTS
    x = np.ascontiguousarray(x, dtype=np.float32)
    w1t = np.ascontiguousarray(w1.T, dtype=np.float32)
    b1c = np.ascontiguousarray(np.asarray(b1, dtype=np.float32).reshape(16, 1))
    w2t = np.ascontiguousarray(w2.T, dtype=np.float32)
    b2r = np.ascontiguousarray(np.asarray(b2, dtype=np.float32).reshape(1, 3))
    in_maps = [
        {"x": x[i], "w1t": w1t, "b1": b1c, "w2t": w2t, "b2": b2r}
        for i in range(B)
    ]
    nc = _get_nc()
    try:
        res = run_bass_kernel_spmd(nc, in_maps, core_ids=list(range(B)), trace=True)
    except Exception:
        res = run_bass_kernel_spmd(nc, in_maps, core_ids=list(range(B)), trace=False)
    LAST_EXEC_TIME_NS = res.exec_time_ns
    LAST_RESULTS = res
    out = np.stack([res.results[i]["out"] for i in range(B)], axis=0)
    return out.astype(np.float32, copy=False)
